# revision 1
# baseline (speedup 1.0000x reference)
"""Trainium2 Bass kernel for nn_ConvDecoder (RBF set-conv decoder).

Reference computation:
    rbf[b,t,g] = exp(-0.5*((x_grid[g]-x_target[b,t])/exp(sigma))^2)
    z[b,t,c]   = sum_g rbf[b,t,g] * r[b,c,g]
    out        = z @ W + b_lin                       # (4, 4096, 2)

Dense evaluation needs 4*4096*8192 = 134M exp() calls. The Gaussian kernel
matrix K_tg is numerically low rank, so we use a Nystrom factorization
through m=64 uniform anchor points u:

    K_tg ~= K_tu @ pinv(K_uu) @ K_ug

The ill-conditioned pinv(K_uu) is folded on the host (fp64) into the
grid-side factor  EguM = K_gu @ pinv(K_uu)  whose entries are bounded
cardinal functions, so the on-device pipeline is fp16/fp32:

    per core (batch b = k//2, target half h = k%2, T = 2048 targets):
      S^T  = r^T-chunks vs EguM-chunks      (c, m)  64 accumulating matmuls K=128
      P    = S @ W   (one matmul)           (m, 2); augmented with b_lin row
      E_ut = exp(u x_t / s^2 + a_u + b_t)   (m+1, T) rank-2 matmul + 1 ACT exp
             (anchor m is a dummy with exponent 0 -> constant 1 row)
      out  = E_ut-slices^T @ P              (t-parts, 2) 16 matmuls -> DMA

Approximation error vs fp64 exact: ~6e-4 relative (tolerance-dominated by
fp16 storage of the factors; the Nystrom error itself is ~4e-7).
"""

import sys

if "/opt/trn_rl_repo" not in sys.path:
    sys.path.insert(0, "/opt/trn_rl_repo")

import numpy as np

# Problem shapes (hardcoded per spec)
B = 4          # batch
C = 64         # conv channels
G = 8192       # grid points
TFULL = 4096   # targets per batch
NCORES = 8
T = B * TFULL // NCORES   # 2048 targets per core
JC = G // 128             # 64 grid chunks of 128
M = 64                    # Nystrom anchors
MA = M + 1                # + dummy "ones" anchor (folds b_lin add)
NSPLIT = 2                # DMA splits for the big tensors
TQ = T // 128             # 16 target chunks of 128
OUT_CH = 2

_PROGRAM = None


def _declare_io(nc, mybir):
    f32 = mybir.dt.float32
    f16 = mybir.dt.float16
    return {
        "egu": nc.dram_tensor("egu", [128, JC, M], f16, kind="ExternalInput"),
        "rt": nc.dram_tensor("rt", [128, JC, C], f16, kind="ExternalInput"),
        "lu": nc.dram_tensor("lu", [2, MA], f32, kind="ExternalInput"),
        "rhs_t": nc.dram_tensor("rhs_t", [2, T], f32, kind="ExternalInput"),
        "ab": nc.dram_tensor("ab", [MA, 1], f32, kind="ExternalInput"),
        "wa": nc.dram_tensor("wa", [C + 1, OUT_CH], f32, kind="ExternalInput"),
        "out": nc.dram_tensor("out", [128, TQ, OUT_CH], f32, kind="ExternalOutput"),
    }


def _load_consts(nc, mybir, dr, constp):
    # consts go on the scalar-engine HWDGE ring so they don't queue ahead of
    # the big data transfers on the sync ring
    f32 = mybir.dt.float32
    lu_sb = constp.tile([2, MA], f32, tag="lu")
    nc.scalar.dma_start(lu_sb[:], dr["lu"][:])
    rhs_sb = constp.tile([2, T], f32, tag="rhs")
    nc.scalar.dma_start(rhs_sb[:], dr["rhs_t"][:])
    ab_sb = constp.tile([MA, 1], f32, tag="ab")
    nc.scalar.dma_start(ab_sb[:], dr["ab"][:])
    wa_sb = constp.tile([C + 1, OUT_CH], f32, tag="wa")
    nc.scalar.dma_start(wa_sb[:], dr["wa"][:])
    return lu_sb, rhs_sb, ab_sb, wa_sb


def _emit_iteration(nc, mybir, dr, consts, datap, midp, psA, psB):
    f32 = mybir.dt.float32
    f16 = mybir.dt.float16
    Exp = mybir.ActivationFunctionType.Exp
    lu_sb, rhs_sb, ab_sb, wa_sb = consts

    # big data first on the sync ring, split so matmuls can start early
    JS = JC // NSPLIT
    egu_t = []
    rt_t = []
    for q in range(NSPLIT):
        e = datap.tile([128, JS, M], f16, tag=f"egu{q}")
        nc.sync.dma_start(e[:], dr["egu"][:, q * JS : (q + 1) * JS, :])
        egu_t.append(e)
        rr = datap.tile([128, JS, C], f16, tag=f"rt{q}")
        nc.sync.dma_start(rr[:], dr["rt"][:, q * JS : (q + 1) * JS, :])
        rt_t.append(rr)

    # E_ut[i,t] = exp(u_i*x_t/s^2 + a_i + b_t); row M is the dummy ones row.
    # Split in halves (2 PSUM banks each, double-buffered) so the exp of one
    # half overlaps the matmuls of the other and reps pipeline.
    TH = T // 2
    eut_sb = midp.tile([MA, T], f16, tag="eut")
    for h in range(2):
        eut_ps = psA.tile([MA, TH], f32, tag="eutp")
        for n in range(TH // 512):
            nc.tensor.matmul(
                eut_ps[:, n * 512 : (n + 1) * 512],
                lu_sb[:],
                rhs_sb[:, h * TH + n * 512 : h * TH + (n + 1) * 512],
                start=True,
                stop=True,
            )
        nc.scalar.activation(
            eut_sb[:, h * TH : (h + 1) * TH], eut_ps[:], Exp, bias=ab_sb[:], scale=1.0
        )

    # S^T[c,i] = sum_g r^T[g,c] * EguM[g,i] : accumulate over 64 chunks (K=128)
    st_ps = psB.tile([C, M], f32, tag="st")
    for q in range(NSPLIT):
        for jj in range(JS):
            j = q * JS + jj
            nc.tensor.matmul(
                st_ps[:],
                rt_t[q][:, jj, :],
                egu_t[q][:, jj, :],
                start=(j == 0),
                stop=(j == JC - 1),
            )
    st_sb = midp.tile([C, M], f32, tag="st")
    nc.scalar.copy(st_sb[:], st_ps[:])

    # P = S @ W -> (m, 2); p_sb row M holds b_lin (from wa row C)
    p_ps = psB.tile([M, OUT_CH], f32, tag="p")
    nc.tensor.matmul(p_ps[:], st_sb[:], wa_sb[0:C, :], start=True, stop=True)
    p_sb = midp.tile([MA, OUT_CH], f16, tag="p")
    nc.scalar.copy(p_sb[0:M, :], p_ps[:])
    nc.vector.tensor_copy(p_sb[M : M + 1, :], wa_sb[C : C + 1, :])

    # out[t,o] = sum_i E_ut[i,t] * P[i,o]  -> (128, TQ*2) psum, one bank
    v_ps = psB.tile([128, TQ * OUT_CH], f32, tag="v")
    for q in range(TQ):
        nc.tensor.matmul(
            v_ps[:, q * OUT_CH : (q + 1) * OUT_CH],
            eut_sb[:, q * 128 : (q + 1) * 128],
            p_sb[:],
            start=True,
            stop=True,
        )
    out_sb = midp.tile([128, TQ * OUT_CH], f32, tag="o")
    nc.scalar.copy(out_sb[:], v_ps[:])
    nc.scalar.dma_start(
        dr["out"][:, :, :], out_sb[:].rearrange("p (q o) -> p q o", o=OUT_CH)
    )


def _build_program(reps=1, loop_iters=None):
    import concourse.bass as bass
    import concourse.tile as tile
    from concourse import bacc, mybir

    nc = bacc.Bacc(None, target_bir_lowering=False)
    dr = _declare_io(nc, mybir)

    with tile.TileContext(nc) as tc:
        with (
            tc.tile_pool(name="const", bufs=1) as constp,
            tc.tile_pool(name="data", bufs=2) as datap,
            tc.tile_pool(name="mid", bufs=2) as midp,
            tc.tile_pool(name="psA", bufs=2, space=bass.MemorySpace.PSUM) as psA,
            tc.tile_pool(name="psB", bufs=1, space=bass.MemorySpace.PSUM) as psB,
        ):
            consts = _load_consts(nc, mybir, dr, constp)
            if loop_iters is not None:
                with tc.For_i(0, loop_iters, 1):
                    for _ in range(reps):
                        _emit_iteration(nc, mybir, dr, consts, datap, midp, psA, psB)
            else:
                for _ in range(reps):
                    _emit_iteration(nc, mybir, dr, consts, datap, midp, psA, psB)

    nc.compile()
    return nc


def _get_program():
    global _PROGRAM
    if _PROGRAM is None:
        _PROGRAM = _build_program()
    return _PROGRAM


def kernel(r, x_context, y_context, x_target, x_grid, sigma, W, b_lin):
    from concourse.bass_utils import run_bass_kernel_spmd

    r = np.asarray(r, dtype=np.float32)
    xt_all = np.asarray(x_target, dtype=np.float64)[..., 0]       # (B, TFULL)
    xg = np.asarray(x_grid, dtype=np.float64)[:, 0]               # (G,)
    s = float(np.exp(np.float64(np.asarray(sigma).reshape(-1)[0])))
    W = np.asarray(W, dtype=np.float64)
    b_lin = np.asarray(b_lin, dtype=np.float64)

    # ---- host-side Nystrom factor prep (all O(G*M), fp64) ----
    lo = min(xg.min(), xt_all.min()) - 3.0 * s
    hi = max(xg.max(), xt_all.max()) + 3.0 * s
    u = np.linspace(lo, hi, M)
    Kuu = np.exp(-0.5 * ((u[:, None] - u[None, :]) / s) ** 2)
    Minv = np.linalg.pinv(Kuu, rcond=1e-10)
    EguM = np.exp(-0.5 * ((xg[:, None] - u[None, :]) / s) ** 2) @ Minv  # (G, M)
    egu_host = np.ascontiguousarray(
        EguM.astype(np.float16).reshape(JC, 128, M).transpose(1, 0, 2)
    )  # (128, JC, M)

    inv_s2 = 1.0 / (s * s)
    # anchor M is a dummy: zero coefficients + zero bias -> exp(0) = 1
    lu_host = np.zeros((2, MA), dtype=np.float32)
    lu_host[0, :M] = u * inv_s2
    lu_host[1, :M] = 1.0
    ab_host = np.zeros((MA, 1), dtype=np.float32)
    ab_host[:M, 0] = -0.5 * u * u * inv_s2
    wa_host = np.ascontiguousarray(
        np.concatenate([W, b_lin[None, :]], axis=0).astype(np.float32)
    )  # (C+1, 2)

    in_maps = []
    for k in range(NCORES):
        b, h = divmod(k, 2)
        rt_host = np.ascontiguousarray(
            r[b].T.astype(np.float16).reshape(JC, 128, C).transpose(1, 0, 2)
        )  # (128, JC, C)
        xt = xt_all[b, h * T : (h + 1) * T]  # (T,)
        rhs_host = np.ascontiguousarray(
            np.stack([xt, -0.5 * xt * xt * inv_s2]).astype(np.float32)
        )  # (2, T)
        in_maps.append(
            {
                "egu": egu_host,
                "rt": rt_host,
                "lu": lu_host,
                "rhs_t": rhs_host,
                "ab": ab_host,
                "wa": wa_host,
            }
        )

    nc = _get_program()
    res = run_bass_kernel_spmd(nc, in_maps, core_ids=list(range(NCORES)))

    out = np.empty((B, TFULL, OUT_CH), dtype=np.float32)
    for k in range(NCORES):
        b, h = divmod(k, 2)
        # device out layout: [p, q, o] -> target index q*128+p
        out[b, h * T : (h + 1) * T] = (
            res.results[k]["out"].transpose(1, 0, 2).reshape(T, OUT_CH)
        )
    return out



# revision 3
# speedup vs baseline: 1.8706x; 1.8706x over previous
"""Trainium2 Bass kernel for nn_ConvDecoder (RBF set-conv decoder).

Reference computation:
    rbf[b,t,g] = exp(-0.5*((x_grid[g]-x_target[b,t])/exp(sigma))^2)
    z[b,t,c]   = sum_g rbf[b,t,g] * r[b,c,g]
    out        = z @ W + b_lin                       # (4, 4096, 2)

The Gaussian kernel matrix K_tg is numerically low rank; a Nystrom
factorization through m=32 uniform anchors u (host-folded pinv(Kuu)
into bounded cardinal functions EguM = K_gu @ pinv(Kuu)) gives

    K_tg ~= E_tu @ EguM^T        (error ~5e-4 at fp16 storage)

Sharding: core k = (batch b = k//2, grid half gh = k%2). Each core
contracts its half of the grid and produces a PARTIAL output for all
4096 targets of its batch; the host sums the two halves and adds b_lin.
This halves per-core HBM traffic vs a target-split (only 0.75 MB/core).

Per-core device pipeline (T=4096 targets, 4096 grid rows):
  args = lhsT.T @ rhs   K=28 fp16 matmul -> (128, 1024) PSUM fp32
         (4 target-quarters packed on partitions: row 32*jq+u covers
          anchor u / target quarter jq; fp32 accuracy recovered via
          hi/lo-split fp16 rows, since fp16 products accumulate
          exactly in fp32 PSUM)
  eut  = exp(args)      one ACT call -> (128, 1024) f16
  S^T  = sum_j rt_j^T @ egu_j   32 accumulating K=128 matmuls -> (64, 32)
  P    = S @ W          4 matmuls into block-diagonal (128, 8) layout
  out  = eut-chunk^T @ P_blk    8 K=128 matmuls -> (128, 64) -> DMA

All big operands ride ONE merged DRAM tensor (128, 32, 96) f16
(egu | rt interleaved per 128-row grid chunk) in 2 DMA slices; all
small operands ride ONE (64, 1162) f16 const tensor.
"""

import sys

if "/opt/trn_rl_repo" not in sys.path:
    sys.path.insert(0, "/opt/trn_rl_repo")

import numpy as np

# Problem shapes (hardcoded per spec)
B = 4          # batch
C = 64         # conv channels
G = 8192       # grid points
TFULL = 4096   # targets per batch (all handled by each core)
NCORES = 8
GH = G // 2            # grid rows per core
JC = GH // 128         # 32 grid chunks of 128
M = 32                 # Nystrom anchors
NQ = 4                 # target quarters packed on partitions (4*32=128)
TQ = TFULL // NQ       # 1024 targets per quarter = eut cols
KROWS = 7 * NQ         # 28 fp16 arg rows (hi/lo split)
OUT_CH = 2
NSPLIT = 2             # DMA slices for the big tensor
CCHUNK = TQ // 128     # 8 final-contraction chunks
MARGIN = 2.0           # anchor span margin in units of s

CST_COLS = TQ + 128 + OUT_CH   # rhs | lhsT | wa

_PROGRAM = None


def _build_program():
    import concourse.bass as bass
    import concourse.tile as tile
    from concourse import bacc, mybir

    f32 = mybir.dt.float32
    f16 = mybir.dt.float16
    Exp = mybir.ActivationFunctionType.Exp

    nc = bacc.Bacc(None, target_bir_lowering=False)
    dr_big = nc.dram_tensor("big", [128, JC, M + C], f16, kind="ExternalInput")
    dr_cst = nc.dram_tensor("cst", [C, CST_COLS], f16, kind="ExternalInput")
    dr_out = nc.dram_tensor("out", [128, CCHUNK * NQ * OUT_CH], f32,
                            kind="ExternalOutput")

    with tile.TileContext(nc) as tc:
        with (
            tc.tile_pool(name="sb", bufs=1) as sbp,
            tc.tile_pool(name="ps", bufs=1, space=bass.MemorySpace.PSUM) as psp,
        ):
            # ---- DMAs in ----
            cst = sbp.tile([C, CST_COLS], f16, tag="cst")
            nc.scalar.dma_start(cst[:], dr_cst[:])

            big = sbp.tile([128, JC, M + C], f16, tag="big")
            JS = JC // NSPLIT
            for q in range(NSPLIT):
                nc.sync.dma_start(
                    big[:, q * JS : (q + 1) * JS, :],
                    dr_big[:, q * JS : (q + 1) * JS, :],
                )

            a_rhs = cst[0:KROWS, 0:TQ]
            a_lhsT = cst[0:KROWS, TQ : TQ + 128]
            wa = cst[0:C, TQ + 128 : TQ + 128 + OUT_CH]

            # ---- eut = exp(args), 4 quarters packed on partitions ----
            args_ps = psp.tile([128, TQ], f32, tag="args")
            for n in range(TQ // 512):
                nc.tensor.matmul(
                    args_ps[:, n * 512 : (n + 1) * 512],
                    a_lhsT,
                    a_rhs[:, n * 512 : (n + 1) * 512],
                    start=True,
                    stop=True,
                )
            eut = sbp.tile([128, TQ], f16, tag="eut")
            nc.scalar.activation(eut[:], args_ps[:], Exp)

            # ---- S^T[c,u] accumulated over 32 grid chunks ----
            st_ps = psp.tile([C, M], f32, tag="st")
            for j in range(JC):
                nc.tensor.matmul(
                    st_ps[:],
                    big[:, j, M : M + C],
                    big[:, j, 0:M],
                    start=(j == 0),
                    stop=(j == JC - 1),
                )
            st_sb = sbp.tile([C, M], f16, tag="st")
            nc.vector.tensor_copy(st_sb[:], st_ps[:])

            # ---- P = S @ W into block-diagonal (128, 8) f16 ----
            p_blk = sbp.tile([128, NQ * OUT_CH], f16, tag="pblk")
            nc.vector.memset(p_blk[:], 0)
            pb_ps = psp.tile([128, NQ * OUT_CH], f32, tag="pb")
            for jq in range(NQ):
                nc.tensor.matmul(
                    pb_ps[32 * jq : 32 * (jq + 1), 2 * jq : 2 * (jq + 1)],
                    st_sb[:],
                    wa,
                    start=True,
                    stop=True,
                    tile_position=(0, 32 * jq),
                )
                nc.vector.tensor_copy(
                    p_blk[32 * jq : 32 * (jq + 1), 2 * jq : 2 * (jq + 1)],
                    pb_ps[32 * jq : 32 * (jq + 1), 2 * jq : 2 * (jq + 1)],
                )

            # ---- partial out: (128, 8) per 128-col eut chunk ----
            v_ps = psp.tile([128, CCHUNK * NQ * OUT_CH], f32, tag="v")
            NW = NQ * OUT_CH
            for cch in range(CCHUNK):
                nc.tensor.matmul(
                    v_ps[:, cch * NW : (cch + 1) * NW],
                    eut[:, cch * 128 : (cch + 1) * 128],
                    p_blk[:],
                    start=True,
                    stop=True,
                )
            out_sb = sbp.tile([128, CCHUNK * NW], f32, tag="o")
            nc.vector.tensor_copy(out_sb[:], v_ps[:])
            nc.scalar.dma_start(dr_out[:], out_sb[:])

    nc.compile()
    return nc


def _get_program():
    global _PROGRAM
    if _PROGRAM is None:
        _PROGRAM = _build_program()
    return _PROGRAM


def _f16(a):
    return a.astype(np.float16)


def kernel(r, x_context, y_context, x_target, x_grid, sigma, W, b_lin):
    from concourse.bass_utils import run_bass_kernel_spmd

    r = np.asarray(r, dtype=np.float32)
    xt_all = np.asarray(x_target, dtype=np.float64)[..., 0]       # (B, TFULL)
    xg = np.asarray(x_grid, dtype=np.float64)[:, 0]               # (G,)
    s = float(np.exp(np.float64(np.asarray(sigma).reshape(-1)[0])))
    W64 = np.asarray(W, dtype=np.float64)
    b_lin = np.asarray(b_lin, dtype=np.float64)

    # ---- host-side Nystrom factor prep (O(G*M), fp64) ----
    lo = min(xg.min(), xt_all.min()) - MARGIN * s
    hi = max(xg.max(), xt_all.max()) + MARGIN * s
    u = np.linspace(lo, hi, M)
    inv_s2 = 1.0 / (s * s)
    Kuu = np.exp(-0.5 * ((u[:, None] - u[None, :]) / s) ** 2)
    Minv = np.linalg.pinv(Kuu, rcond=1e-10)
    EguM = np.exp(-0.5 * ((xg[:, None] - u[None, :]) / s) ** 2) @ Minv  # (G, M)
    egu16 = _f16(EguM)

    # anchor-side hi/lo rows (shared across batches)
    uc = u * inv_s2
    uch = _f16(uc)
    ucl = _f16(uc - uch.astype(np.float64))
    a_u = -0.5 * u * u * inv_s2
    ah = _f16(a_u)
    al = _f16(a_u - ah.astype(np.float64))

    cst_by_batch = []
    for b in range(B):
        x = xt_all[b]
        bt = -0.5 * x * x * inv_s2
        xh = _f16(x)
        xl = _f16(x - xh.astype(np.float64))
        bh = _f16(bt)
        bl = _f16(bt - bh.astype(np.float64))
        cst = np.zeros((C, CST_COLS), dtype=np.float16)
        for jq in range(NQ):
            base = 7 * jq
            sl = slice(jq * TQ, (jq + 1) * TQ)
            # rhs region: cols 0:TQ
            cst[base + 0, 0:TQ] = xh[sl]
            cst[base + 1, 0:TQ] = xl[sl]
            cst[base + 2, 0:TQ] = xh[sl]
            cst[base + 3, 0:TQ] = bh[sl]
            cst[base + 4, 0:TQ] = bl[sl]
            cst[base + 5, 0:TQ] = 1.0
            cst[base + 6, 0:TQ] = 1.0
            # lhsT region: cols TQ:TQ+128, partition block jq
            pcols = slice(TQ + 32 * jq, TQ + 32 * (jq + 1))
            cst[base + 0, pcols] = uch
            cst[base + 1, pcols] = uch
            cst[base + 2, pcols] = ucl
            cst[base + 3, pcols] = 1.0
            cst[base + 4, pcols] = 1.0
            cst[base + 5, pcols] = ah
            cst[base + 6, pcols] = al
        cst[0:C, TQ + 128 : TQ + 128 + OUT_CH] = _f16(W64)
        cst_by_batch.append(np.ascontiguousarray(cst))

    in_maps = []
    for k in range(NCORES):
        b, gh = divmod(k, 2)
        gsl = slice(gh * GH, (gh + 1) * GH)
        big = np.empty((128, JC, M + C), dtype=np.float16)
        big[:, :, 0:M] = egu16[gsl].reshape(JC, 128, M).transpose(1, 0, 2)
        big[:, :, M:] = _f16(r[b].T[gsl]).reshape(JC, 128, C).transpose(1, 0, 2)
        in_maps.append({"big": np.ascontiguousarray(big), "cst": cst_by_batch[b]})

    nc = _get_program()
    res = run_bass_kernel_spmd(nc, in_maps, core_ids=list(range(NCORES)))

    out = np.empty((B, TFULL, OUT_CH), dtype=np.float32)
    for b in range(B):
        acc = None
        for gh in range(2):
            v = res.results[2 * b + gh]["out"].reshape(128, CCHUNK, NQ, OUT_CH)
            part = v.transpose(2, 1, 0, 3).reshape(TFULL, OUT_CH)
            acc = part if acc is None else acc + part
        out[b] = acc
    out += b_lin.astype(np.float32)[None, None, :]
    return out
